# revision 87
# baseline (speedup 1.0000x reference)
"""Trainium2 Bass kernel for a single-layer transformer encoder.

Model: B=2, N=2048, D=1024, H=16, DFF=4096 (pre-computed QKV attention +
residual/LN + GELU FFN + residual/LN).

Sharding: 2 batches x 4-way sequence split (core c owns the 512 query
tokens q=c%4 of batch b=c//4).  Each core computes the FULL K/V for its
batch (2x redundant vs a pair-split, but it removes every cross-core
collective/exchange and their latency cliffs; the extra PE work rides in
slack).  Rotated token quarters order production q0,q2,q1,q3.

Attention is emitted as eight sequential head-pair sweeps JIT-interleaved
with production: exp(hp, quarter t) needs only K feature chunk kf=hp of t,
Q chunk hp, and V dvc=hp//4 of t, so scores/exp start ~5us in while the
remaining K feature chunks and the V dvc=1 half are produced between
sweeps.  Engine use is balanced so the ~133us of ScalarE exp ([128,1024]
PSUM tiles, denominators from a ones-column in V) and ~140us of PE work
overlap at >80% each.  PSUM is budgeted exactly: pt double-buffer 8KB +
one sweep's oaccs 4KB + production/bc accs 4KB = 16KB.

All dense projections (QKV, attention output, FFN1/FFN2) run as fp8-e4m3
DoubleRow matmuls.  fp8 weights are pre-scaled x16 on the host: raw
magnitudes (~1/sqrt(fan_in)) sit at/below e4m3's min normal where
quantization is absolute, and the rescale roughly halves the end-to-end
error (compensation folds into exp/gelu scale immediates, pre-scaled
biases/LN1 affine, and LN scale-invariance).

Scheduling details the tile framework's greedy ready-list scheduler needs
help with: ACT_TABLE_LOADs are pulled off the critical path by dummy
Sqrt/Gelu activations dep-pinned right after the last exp / LN1-sqrt /
last gelu; w1 prefetch DMAs are emitted between early sweeps (8-slot pool,
no slot-reuse waits) while w2 prefetches ride the idle SP queue gated by
the freed xT/wall pool boundary; FFN2's first 4 output chunks accumulate
k-partials inside the gelu-paced FFN1 window (2 pspt slots as column-pair
accumulators); LN stats are PE ones-column matmuls interleaved into the
producing loops; LN applies split token-halves across DVE and GpSimd.
Output DMA is pipelined across both HWDGE queues.
"""

import os
import sys

for _p in ("/opt/trn_rl_repo", "/root/.axon_site", "/root/.axon_site/_ro/trn_rl_repo"):
    if os.path.isdir(_p) and _p not in sys.path:
        sys.path.append(_p)

import numpy as np

import concourse.bacc as bacc
import concourse.mybir as mybir
import concourse.tile as tile
from concourse.tile_rust import add_dep_helper
from concourse.bass_utils import run_bass_kernel_spmd

P = 128
B, NSEQ, D, H, DFF = 2, 2048, 1024, 16, 4096
DH = D // H                     # 64
NT = 512                        # query tokens per core
DM = D // P                     # 8 feature chunks
JC = NSEQ // P                  # 16 key-token chunks
TC = NSEQ // 512                # 4 512-token chunks
FC = DFF // P                   # 32 FFN feature chunks
HPAIRS = H // 2                 # 8
SCALE = DH ** -0.5
EPS = 1e-5
WS = 16.0                       # fp8 weight pre-scale (see _tile_w)

F32 = mybir.dt.float32
F32R = mybir.dt.float32r
BF16 = mybir.dt.bfloat16
FP8 = mybir.dt.float8e4
I16 = mybir.dt.int16
# Schraudolph exp-approximation constants (DVE path): bf16 bitcast of
# round(x*SEXP_A + SEXP_B) ~= exp(x * SCALE/WS^2); max rel err 3.3%,
# rms 2.1% (c=5.5), only ever applied to a minority of score chunks.
SEXP_A = (SCALE / (WS * WS)) * float(np.log2(np.e)) * 128.0
SEXP_B = 127.0 * 128.0 - 5.5
SEXP_N = int(os.environ.get("SEXP_N", "0"))   # of every 8 chunks, N on DVE
VCP_ACT = os.environ.get("VCP_ACT", "0") == "1"  # V-proj copies on ACT
DR = mybir.MatmulPerfMode.DoubleRow
AF = mybir.ActivationFunctionType

_NC_CACHE = None


def _rearr(ap):
    """DRAM [D_like, T] -> [p, chunk, T] view with chunk-major features."""
    return ap.rearrange("(c p) t -> p c t", p=P)


def _build_nc(reps=1, phases=("qkv", "attn", "proj", "ffn")):
    nc = bacc.Bacc("TRN2", target_bir_lowering=False, debug=False)
    nc.num_devices = 8

    xT = nc.dram_tensor("xT", [D, NSEQ], FP8, kind="ExternalInput")
    x_own = nc.dram_tensor("x_own", [D, NT], BF16, kind="ExternalInput")
    # weights arrive pre-tiled: [out_chunk, partition, in_chunk, out_cols]
    w_q = nc.dram_tensor("w_q", [DM, P, DM, P], FP8, kind="ExternalInput")
    w_k = nc.dram_tensor("w_k", [DM, P, DM, P], FP8, kind="ExternalInput")
    w_v = nc.dram_tensor("w_v", [2, P, DM, 512], FP8, kind="ExternalInput")
    w_out = nc.dram_tensor("w_out", [DM, P, DM, P], FP8, kind="ExternalInput")
    w1 = nc.dram_tensor("w1", [DFF // 512, P, DM, 512], FP8,
                        kind="ExternalInput")
    w2 = nc.dram_tensor("w2", [DM, P, FC, P], FP8, kind="ExternalInput")
    b1 = nc.dram_tensor("b1", [DFF], F32, kind="ExternalInput")
    b2 = nc.dram_tensor("b2", [D], F32, kind="ExternalInput")
    ln1_w = nc.dram_tensor("ln1_w", [D], F32, kind="ExternalInput")
    ln1_b = nc.dram_tensor("ln1_b", [D], F32, kind="ExternalInput")
    ln2_w = nc.dram_tensor("ln2_w", [D], F32, kind="ExternalInput")
    ln2_b = nc.dram_tensor("ln2_b", [D], F32, kind="ExternalInput")
    yT = nc.dram_tensor("yT", [D, NT], F32, kind="ExternalOutput")

    tensors = dict(xT=xT, x_own=x_own, w_q=w_q, w_k=w_k, w_v=w_v, w_out=w_out, w1=w1,
                   w2=w2, b1=b1, b2=b2, ln1_w=ln1_w, ln1_b=ln1_b,
                   ln2_w=ln2_w, ln2_b=ln2_b, yT=yT)

    with tile.TileContext(nc) as tc, \
         nc.allow_low_precision(reason="bf16 matmul operands; fp32 spine"), \
         tc.tile_pool(name="const", bufs=1) as pc:
        C = _emit_consts(nc, pc, tensors)
        for r in range(reps):
            _emit(nc, tc, tensors, C, phases=phases)
    nc.compile()
    return nc


def _emit_consts(nc, pc, T):
    """Constant tiles, emitted ONCE outside the rep loop: re-emitting them
    per rep creates WAR chains from each rep's first memset back to the
    previous rep's very last constant read, serializing reps."""
    ones_f32 = pc.tile([P, 2 * P], F32)
    nc.vector.memset(ones_f32[:], 1.0)
    ones_col = pc.tile([P, 1], BF16)          # lhsT for bf16 stat sums
    nc.vector.tensor_copy(ones_col[:], ones_f32[:, 0:1])
    ones_row = pc.tile([1, P], F32)           # lhsT for exact broadcasts
    nc.vector.tensor_copy(ones_row[:], ones_f32[0:1, 0:P])
    ones_row_b = pc.tile([1, P], BF16)        # bf16 lhsT (full-rate matmul)
    nc.vector.tensor_copy(ones_row_b[:], ones_f32[0:1, 0:P])
    eps_sb = pc.tile([1, 1], F32)
    nc.vector.memset(eps_sb[:], EPS)
    b1_sb = pc.tile([P, FC], F32)
    b2_sb = pc.tile([P, DM], F32)
    lnw1_sb = pc.tile([P, DM], F32)
    lnb1_sb = pc.tile([P, DM], F32)
    lnw2_sb = pc.tile([P, DM], F32)
    lnb2_sb = pc.tile([P, DM], F32)
    dummy = pc.tile([1, 1], F32)
    nc.vector.memset(dummy[:], 1.0)
    for sb, t in ((b1_sb, "b1"), (b2_sb, "b2"),
                  (lnw1_sb, "ln1_w"), (lnb1_sb, "ln1_b"),
                  (lnw2_sb, "ln2_w"), (lnb2_sb, "ln2_b")):
        nc.gpsimd.dma_start(sb[:], T[t].ap().rearrange("(c p) -> p c", p=P))
    return dict(ones_f32=ones_f32, ones_col=ones_col, ones_row=ones_row,
                ones_row_b=ones_row_b, eps_sb=eps_sb, b1_sb=b1_sb,
                b2_sb=b2_sb, lnw1_sb=lnw1_sb, lnb1_sb=lnb1_sb,
                lnw2_sb=lnw2_sb, lnb2_sb=lnb2_sb, dummy=dummy)


def _emit(nc, tc, T, C, phases=("qkv", "attn", "proj", "ffn")):
    xT_d, yT_d = T["xT"], T["yT"]
    ones_f32 = C["ones_f32"]
    ones_col = C["ones_col"]
    ones_row = C["ones_row"]
    ones_row_b = C["ones_row_b"]
    eps_sb = C["eps_sb"]
    b1_sb = C["b1_sb"]
    b2_sb = C["b2_sb"]
    lnw1_sb = C["lnw1_sb"]
    lnb1_sb = C["lnb1_sb"]
    lnw2_sb = C["lnw2_sb"]
    lnb2_sb = C["lnb2_sb"]
    dummy = C["dummy"]

    # ---------------- whole-kernel pools ----------------
    with tc.tile_pool(name="pers", bufs=1) as pers, \
         tc.tile_pool(name="scratch", bufs=4) as sq_pool, \
         tc.tile_pool(name="vecs", bufs=4) as vec_pool, \
         tc.tile_pool(name="psacc", bufs=2, space="PSUM") as psacc, \
         tc.tile_pool(name="pspt", bufs=2, space="PSUM") as pspt, \
         tc.tile_pool(name="psout", bufs=2, space="PSUM") as psout:

        # persistent activations
        QT = pers.tile([P, DM, NT], BF16)
        outT = pers.tile([P, DM, NT], FP8)
        xow = pers.tile([P, DM, NT], BF16, tag="tc")
        xln18 = pers.tile([P, DM, NT], FP8)  # fp8 operand copy of xln1  # own-token x (residual 1)
        xln1 = pers.tile([P, DM, NT], BF16)     # LN1 out (ffn operand+residual)

        def ln_apply(z_tile, writes, interleave=None):
            """LayerNorm over features of z_tile [P, DM, NT] (fp32).
            writes(k, src_f32_ap) stores chunk k.  If interleave is given,
            (s1, s2) stats were already accumulated there by the caller."""
            if interleave is None:
                s1 = psacc.tile([1, NT], F32, tag="acc")
                s2 = psacc.tile([1, NT], F32, tag="acc")
                for k in range(DM):
                    eng = nc.vector if k % 2 == 0 else nc.gpsimd
                    nc.tensor.matmul(s1[:], ones_col[:], z_tile[:, k, :],
                                     start=(k == 0), stop=(k == DM - 1))
                    sq = sq_pool.tile([P, NT], BF16, tag="sq")
                    eng.tensor_mul(sq[:], z_tile[:, k, :], z_tile[:, k, :])
                    nc.tensor.matmul(s2[:], ones_col[:], sq[:],
                                     start=(k == 0), stop=(k == DM - 1))
            else:
                s1, s2 = interleave
            mu = vec_pool.tile([1, NT], F32, tag="v")
            nc.vector.tensor_scalar_mul(mu[:], s1[:], 1.0 / D)
            var = vec_pool.tile([1, NT], F32, tag="v")
            nc.vector.tensor_mul(var[:], mu[:], s1[:])
            nc.vector.tensor_sub(var[:], s2[:], var[:])
            sqrt_ins = nc.scalar.activation(var[:], var[:], AF.Sqrt,
                                            scale=1.0 / D, bias=eps_sb[:])
            ln_apply.last_sqrt = sqrt_ins
            rec = vec_pool.tile([1, NT], F32, tag="v")
            nc.vector.reciprocal(rec[:], var[:])
            murf = vec_pool.tile([1, NT], F32, tag="v")
            nc.vector.tensor_mul(murf[:], mu[:], rec[:])
            R = psacc.tile([P, NT], F32, tag="acc")
            nc.tensor.matmul(R[:], ones_row[:], rec[:],
                             start=True, stop=True)
            MR = psacc.tile([P, NT], F32, tag="acc")
            nc.tensor.matmul(MR[:], ones_row[:], murf[:],
                             start=True, stop=True)
            R_sb = vec_pool.tile([P, NT], F32, tag="v")
            nc.scalar.copy(R_sb[:], R[:])
            MR_sb = vec_pool.tile([P, NT], F32, tag="v")
            nc.scalar.copy(MR_sb[:], MR[:])
            HNT = NT // 2
            for k in range(DM):
                for h in range(2):
                    eng = nc.vector if h == 0 else nc.gpsimd
                    cols = slice(h * HNT, (h + 1) * HNT)
                    t = sq_pool.tile([P, HNT], F32, tag="sq")
                    eng.tensor_mul(t[:], z_tile[:, k, cols], R_sb[:, cols])
                    eng.tensor_sub(t[:], t[:], MR_sb[:, cols])
                    writes(k, cols, t, eng)

        with tc.tile_pool(name="ktp", bufs=1) as kt_pool, \
             tc.tile_pool(name="vpp", bufs=1) as vp_pool, \
             tc.tile_pool(name="pt", bufs=6) as pt_pool, \
             tc.tile_pool(name="wop", bufs=1) as wo_pool, \
             tc.tile_pool(name="w1p", bufs=8) as w1_pool:
            wo_t = wo_pool.tile([P, DM, DM, P], FP8)
            w1ts = []

            def prefetch_w1(lo, hi):
                # Pool-queue DMAs, interleaved between attention sweeps so
                # they run early but never starve a fin's broadcast.
                for fg in range(lo, hi):
                    w1t = w1_pool.tile([P, DM, 512], FP8, tag="w1",
                                       name=f"w1t{fg}")
                    nc.gpsimd.dma_start(w1t[:], T["w1"].ap()[fg])
                    w1ts.append(w1t)

            kt = kt_pool.tile([P, DM, 1024], BF16, tag="kt")
            ktr = kt_pool.tile([P, DM, 1024], BF16, tag="ktr")
            vp = vp_pool.tile([P, 8, H * 65], BF16, tag="vp")
            vpr = vp_pool.tile([P, 8, H * 65], BF16, tag="vpr")
            vp_h = vp.rearrange("p j (h e) -> p j h e", e=65)
            vpr_h = vpr.rearrange("p j (h e) -> p j h e", e=65)
            nc.vector.tensor_copy(
                vp_h[:, :, :, 64:65],
                ones_f32[:, 0:128].rearrange("p (a b c) -> p a b c", b=H, c=1))
            nc.vector.tensor_copy(
                vpr_h[:, :, :, 64:65],
                ones_f32[:, 0:128].rearrange("p (a b c) -> p a b c",
                                             b=H, c=1))

            def kt_at(jc):
                """(tile, column-base) for rotated key chunk jc."""
                if jc < 4: return kt, jc * P
                if jc < 8: return ktr, (jc - 4) * P
                if jc < 12: return kt, (jc - 4) * P
                return ktr, (jc - 8) * P

            def vp_at(jc):
                if jc < 4: return vp, jc
                if jc < 8: return vpr, jc - 4
                if jc < 12: return vp, jc - 4
                return vpr, jc - 8

            def vp_at_h(jc):
                if jc < 4: return vp_h, jc
                if jc < 8: return vpr_h, jc - 4
                if jc < 12: return vp_h, jc - 4
                return vpr_h, jc - 8

            # ---------- attention helpers (emission interleaved below) ----
            oaccs = {}
            last_exp = [None]
            exp_ctr = [0]

            def attn_chunk(hp, jcs, pool, tag, start, stop):
                if start:
                    oaccs[hp] = [pool.tile([65, NT], F32, tag=tag,
                                           name=f"oacc{hp}_{i}")
                                 for i in range(2)]
                oacc = oaccs[hp]
                for n, jc in enumerate(jcs):
                    ksrc, kcb = kt_at(jc)
                    vsrc, vpos = vp_at(jc)
                    pt_ps = pspt.tile([P, 2 * NT], F32, tag="pt")
                    for i in range(2):
                        rows = slice(64 * i, 64 * i + 64)
                        nc.tensor.matmul(
                            pt_ps[:, i * NT:(i + 1) * NT],
                            ksrc[rows, hp, kcb:kcb + P],
                            QT[rows, hp, :],
                            start=True, stop=True)
                    pt_sb = pt_pool.tile([P, 2 * NT], BF16, tag="ptsb")
                    exp_ctr[0] += 1
                    if exp_ctr[0] % 8 < SEXP_N:
                        # DVE Schraudolph exp: offloads the ACT engine (the
                        # attention-phase co-bottleneck) for 3/8 of chunks.
                        nc.vector.tensor_scalar(
                            pt_sb[:].bitcast(I16), pt_ps[:],
                            SEXP_A, SEXP_B,
                            op0=mybir.AluOpType.mult,
                            op1=mybir.AluOpType.add)
                    else:
                        last_exp[0] = nc.scalar.activation(
                            pt_sb[:], pt_ps[:], AF.Exp,
                            scale=SCALE / (WS * WS))
                    for i in range(2):
                        h = 2 * hp + i
                        nc.tensor.matmul(
                            oacc[i][:],
                            vsrc[:, vpos, h * 65:(h + 1) * 65],
                            pt_sb[:, i * NT:(i + 1) * NT],
                            start=(start and n == 0),
                            stop=(stop and n == len(jcs) - 1))

            def attn_fin(hp):
                oacc = oaccs.pop(hp)
                bc2 = psacc.tile([P, NT], F32, tag="acc")
                for i in range(2):
                    # bf16 denominators: 0.4% relative, swamped by the fp8
                    # outT quantization right after; bf16 matmul runs at
                    # full rate and, unlike f32r, may target partition 64.
                    rec = vec_pool.tile([1, NT], BF16, tag="v")
                    nc.vector.reciprocal(rec[:], oacc[i][64:65, :])
                    nc.tensor.matmul(bc2[64 * i:64 * i + 64, :],
                                     ones_row_b[:, 0:64], rec[:],
                                     start=True, stop=True)
                bc_sb = sq_pool.tile([P, NT], F32, tag="sq")
                nc.vector.tensor_copy(bc_sb[:], bc2[:])
                for i in range(2):
                    nc.vector.tensor_mul(
                        outT[64 * i:64 * i + 64, hp, :],
                        oacc[i][0:64, :],
                        bc_sb[64 * i:64 * i + 64, :])

            with tc.tile_pool(name="xpool", bufs=1) as px:
                xT = px.tile([P, DM, NSEQ], FP8)
                xTs = _rearr(xT_d.ap())
                xTq = xTs.rearrange("p c (h q2 t) -> p c h q2 t", h=2, q2=2)
                xTt = xT.rearrange("p c (h q2 t) -> p c h q2 t", h=2, q2=2)
                # quarter order q0, q2, q1, q3 matches K/V production order
                nc.sync.dma_start(xTt[:, :, 0, 0, :], xTq[:, :, 0, 0, :])

                with tc.tile_pool(name="wall", bufs=1) as wall_pool:
                    # wk on SP right after xT-q0; wq alone on ACT so the
                    # QT copies (ACT) can start ~4us in, pulling the first
                    # exp forward.
                    wk = wall_pool.tile([P, DM, DM, P], FP8, tag="wk")
                    wks = T["w_k"].ap().rearrange("f p k t -> p f k t")
                    nc.sync.dma_start(wk[:, 0:4], wks[:, 0:4])
                    nc.sync.dma_start(wk[:, 4:8], wks[:, 4:8])
                    wq = wall_pool.tile([P, DM, DM, P], FP8, tag="wq")
                    wqs = T["w_q"].ap().rearrange("f p k t -> p f k t")
                    nc.scalar.dma_start(wq[:, 0:4], wqs[:, 0:4])
                    nc.scalar.dma_start(wq[:, 4:8], wqs[:, 4:8])
                    nc.sync.dma_start(xTt[:, :, 1, 0, :], xTq[:, :, 1, 0, :])
                    nc.sync.dma_start(xTt[:, :, 0, 1, :], xTq[:, :, 0, 1, :])
                    nc.sync.dma_start(xTt[:, :, 1, 1, :], xTq[:, :, 1, 1, :])
                    wos = T["w_out"].ap().rearrange("f p k t -> p f k t")
                    nc.sync.dma_start(wo_t[:, 0:4], wos[:, 0:4])
                    nc.sync.dma_start(wo_t[:, 4:8], wos[:, 4:8])
                    wv = wall_pool.tile([P, 2, DM, 512], FP8, tag="wv")
                    wvs = T["w_v"].ap().rearrange("f p k t -> p f k t")
                    nc.gpsimd.dma_start(wv[:, 0:1], wvs[:, 0:1])
                    nc.gpsimd.dma_start(wv[:, 1:2], wvs[:, 1:2])
                    last_xow_cp = nc.gpsimd.dma_start(
                        xow[:], _rearr(T["x_own"].ap()))

                    def k_quarter(t, kfs):
                        """K^T chunks kfs for rotated token quarter t."""
                        last = None
                        dst, cb = kt_at(t * 4)
                        for kf in kfs:
                            acc = psacc.tile([P, 512], F32, tag="acc")
                            for k in range(DM // 2):
                                last = nc.tensor.matmul(
                                    acc[:], wk[:, kf, 2 * k:2 * k + 2, :],
                                    xT[:, 2 * k:2 * k + 2,
                                       t * 512:(t + 1) * 512],
                                    start=(k == 0), stop=(k == DM // 2 - 1),
                                    perf_mode=DR)
                            nc.vector.tensor_copy(
                                dst[:, kf, cb:cb + 512], acc[:])
                        return last

                    def k_feat(kf):
                        """K^T feature chunk kf (head pair kf), all quarters."""
                        for t in (0, 2, 1, 3):
                            last = k_quarter(t, [kf])
                        return last

                    def v_quarter(t, dvcs):
                        """V for rotated token quarter t, head halves dvcs."""
                        last = None
                        for dvc in dvcs:
                            for jc4 in range(4):
                                jc = t * 4 + jc4
                                acc = psacc.tile([P, 512], F32, tag="acc")
                                for k in range(DM // 2):
                                    last = nc.tensor.matmul(
                                        acc[:],
                                        xT[:, 2 * k:2 * k + 2,
                                           jc * P:(jc + 1) * P],
                                        wv[:, dvc, 2 * k:2 * k + 2, :],
                                        start=(k == 0),
                                        stop=(k == DM // 2 - 1),
                                        perf_mode=DR)
                                dvh, jpos = vp_at_h(jc)
                                dst = dvh[:, jpos, dvc * 8:(dvc + 1) * 8, 0:64]
                                src = acc[:].rearrange("p (h e) -> p h e",
                                                       e=64)
                                if VCP_ACT:
                                    nc.scalar.copy(dst, src)
                                else:
                                    nc.vector.tensor_copy(dst, src)
                        return last

                    # Phase 1 JIT-interleaved with the attention sweeps.
                    # exp(hp, quarter t) needs only K chunk kf=hp of t, Q
                    # chunk hp, and AV needs V dvc=hp//4 of t — so head-pair
                    # sweeps run sequentially while the remaining K feature
                    # chunks / V head-halves are produced between sweeps.
                    JC_QS = ([0, 1, 2, 3], [8, 9, 10, 11],
                             [4, 5, 6, 7], [12, 13, 14, 15])
                    atn = "attn" in phases

                    def sweep(hp, part):  # part 0: quarters q0+q2; 1: q1+q3
                        if not atn:
                            return
                        attn_chunk(hp, JC_QS[2 * part] + JC_QS[2 * part + 1],
                                   psout, "o", start=(part == 0),
                                   stop=(part == 1))
                        if part == 1:
                            attn_fin(hp)

                    def q_proj(qfs):
                        for qf in qfs:
                            acc = psacc.tile([P, NT], F32, tag="acc")
                            for k in range(DM // 2):
                                nc.tensor.matmul(
                                    acc[:], wq[:, qf, 2 * k:2 * k + 2, :],
                                    xT[:, 2 * k:2 * k + 2, 0:NT],
                                    start=(k == 0), stop=(k == DM // 2 - 1),
                                    perf_mode=DR)
                            nc.scalar.copy(QT[:, qf, :], acc[:])

                    k_quarter(0, [0])
                    q_proj(range(DM))
                    v_quarter(0, (0,))
                    if atn:
                        attn_chunk(0, JC_QS[0], psout, "o", True, False)
                    k_quarter(2, [0])
                    v_quarter(2, (0,))
                    if atn:
                        attn_chunk(0, JC_QS[1], psout, "o", False, False)
                    k_quarter(1, [0])
                    v_quarter(1, (0,))
                    if atn:
                        attn_chunk(0, JC_QS[2], psout, "o", False, False)
                    k_quarter(3, [0])
                    v_quarter(3, (0,))
                    if atn:
                        attn_chunk(0, JC_QS[3], psout, "o", False, True)
                    k_feat(1)   # before fin0: its psacc accs beat fin0's bc2
                    if atn:
                        attn_fin(0)
                    if "ffn" in phases:
                        prefetch_w1(0, 4)
                    sweep(1, 0)
                    if "ffn" in phases:
                        prefetch_w1(4, 8)
                    k_feat(2)
                    sweep(1, 1)
                    k_feat(3)
                    sweep(2, 0)
                    v_quarter(0, (1,))
                    sweep(2, 1)
                    v_quarter(2, (1,))
                    sweep(3, 0)
                    v_quarter(1, (1,))
                    k_feat(4)
                    sweep(3, 1)
                    v_quarter(3, (1,))
                    k_feat(5)
                    sweep(4, 0)
                    k_feat(6)
                    sweep(4, 1)
                    k_feat(7)
                    sweep(5, 0)
                    sweep(5, 1)
                    sweep(6, 0)
                    sweep(6, 1)
                    sweep(7, 0)
                    sweep(7, 1)

            # -------- prefetch FFN2 weights (SP queue; reuses the freed
            # xT/wall SBUF region, so the pool boundary gates these DMAs
            # behind the last phase-1 reads) ------------------------------
            _w2cm = tc.tile_pool(name="w2p", bufs=8)
            w2_pool = _w2cm.__enter__()
            w2ts = []
            if "ffn" in phases:
                for ef in range(DM):
                    w2t = w2_pool.tile([P, FC, P], FP8, tag="w2",
                                       name=f"w2t{ef}")
                    nc.sync.dma_start(w2t[:], T["w2"].ap()[ef])
                    w2ts.append(w2t)

            # -------- attention emitted inside phase 1 above -------------
            if "attn" not in phases:      # timing-bisect stub
                for k in range(DM):
                    nc.vector.tensor_copy(outT[:, k, :], QT[:, k, :])

            # -------- output projection + residual 1 + LN1 stats ---------
            # Pull the Sqrt table in right after the last exp, off the
            # critical path (LN1's Sqrt would otherwise eat the 1.28us
            # ACT_TABLE_LOAD inline).  The dep pins it there — the tile
            # scheduler is a greedy ready-list and would otherwise hoist it
            # into an early idle slot where the exps evict it again.
            d1 = nc.scalar.activation(dummy[:], dummy[:], AF.Sqrt)
            if "attn" in phases:
                add_dep_helper(d1.ins, last_exp[0].ins,
                               reason="sqrt preload after last exp")
            z1 = kt_pool.tile([P, DM, NT], BF16, tag="kt")  # reuses kt slot
            s1 = psacc.tile([1, NT], F32, tag="acc")
            s2 = psacc.tile([1, NT], F32, tag="acc")
            for ef in range(DM):
                acc = pspt.tile([P, NT], F32, tag="pt")
                for k in range(DM // 2):
                    nc.tensor.matmul(acc[:], wo_t[:, ef, 2 * k:2 * k + 2, :],
                                     outT[:, 2 * k:2 * k + 2, :],
                                     start=(k == 0),
                                     stop=(k == DM // 2 - 1),
                                     perf_mode=DR)
                if ef % 2 == 0:
                    nc.vector.scalar_tensor_tensor(
                        z1[:, ef, :], acc[:], 1.0 / WS, xow[:, ef, :],
                        op0=mybir.AluOpType.mult, op1=mybir.AluOpType.add)
                else:
                    # DVE is the pacer here: route odd chunks via ACT
                    # (PSUM read + 1/WS scale) + Pool (SBUF add).
                    t = sq_pool.tile([P, NT], F32, tag="sq")
                    nc.scalar.activation(t[:], acc[:], AF.Identity,
                                         scale=1.0 / WS)
                    nc.gpsimd.tensor_add(z1[:, ef, :], t[:], xow[:, ef, :])
                nc.tensor.matmul(s1[:], ones_col[:], z1[:, ef, :],
                                 start=(ef == 0), stop=(ef == DM - 1))
                sq = sq_pool.tile([P, NT], BF16, tag="sq")
                nc.gpsimd.tensor_mul(sq[:], z1[:, ef, :], z1[:, ef, :])
                nc.tensor.matmul(s2[:], ones_col[:], sq[:],
                                 start=(ef == 0), stop=(ef == DM - 1))

            # -------- LN1 ------------------------------------------------
            def write_xln1(k, cols, t, eng):
                eng.tensor_scalar(xln1[:, k, cols], t[:],
                                  lnw1_sb[:, k:k + 1],
                                  lnb1_sb[:, k:k + 1],
                                  op0=mybir.AluOpType.mult,
                                  op1=mybir.AluOpType.add)
                if cols.start != 0:     # second half done: copy whole chunk
                    if k % 2 == 0:
                        nc.scalar.copy(xln18[:, k, :], xln1[:, k, :])
                    else:
                        nc.vector.tensor_copy(xln18[:, k, :], xln1[:, k, :])
            ln_apply(z1, write_xln1, interleave=(s1, s2))
            if "ffn" in phases:   # preload Gelu table behind LN1's Sqrt
                d2 = nc.scalar.activation(dummy[:], dummy[:], AF.Gelu)
                add_dep_helper(d2.ins, ln_apply.last_sqrt.ins,
                               reason="gelu preload after LN1 sqrt")

            if "ffn" not in phases:   # timing-bisect stub: LN2 input
                z2 = pers.tile([P, DM, NT], BF16, tag="tc")  # xow slot
                s1 = psacc.tile([1, NT], F32, tag="acc")
                s2 = psacc.tile([1, NT], F32, tag="acc")
                for k in range(DM):
                    eng = nc.vector if k % 2 == 0 else nc.gpsimd
                    nc.vector.tensor_copy(z2[:, k, :], z1[:, k, :])
                    nc.tensor.matmul(s1[:], ones_col[:], z2[:, k, :],
                                     start=(k == 0), stop=(k == DM - 1))
                    sq = sq_pool.tile([P, NT], BF16, tag="sq")
                    eng.tensor_mul(sq[:], z2[:, k, :], z2[:, k, :])
                    nc.tensor.matmul(s2[:], ones_col[:], sq[:],
                                     start=(k == 0), stop=(k == DM - 1))

            # -------- FFN ------------------------------------------------
            if "ffn" in phases:
                hT = kt_pool.tile([P, FC, NT], FP8, tag="kt")  # kt/z1 slot
                # FFN2 accumulators for ef 0-5 live across FFN1 (2 pspt
                # slots hold 2 accs each in column halves + 2 psout slots):
                # their k-partials are emitted as hT chunk pairs land, so
                # most of FFN2's PE work hides under the gelu-paced FFN1.
                a01 = pspt.tile([P, 2 * NT], F32, tag="pt", name="f2a01")
                a23 = pspt.tile([P, 2 * NT], F32, tag="pt", name="f2a23")
                f2acc = [a01[:, 0:NT], a01[:, NT:2 * NT],
                         a23[:, 0:NT], a23[:, NT:2 * NT]]
                for fg in range(DFF // 512):
                    w1t = w1ts[fg]
                    for f4 in range(4):
                        f = fg * 4 + f4
                        fpool = psacc if f % 2 == 0 else psout
                        ftag = "acc" if f % 2 == 0 else "o"
                        acc = fpool.tile([P, NT], F32, tag=ftag)
                        for k in range(DM // 2):
                            nc.tensor.matmul(
                                acc[:],
                                w1t[:, 2 * k:2 * k + 2, f4 * P:(f4 + 1) * P],
                                xln18[:, 2 * k:2 * k + 2, :],
                                start=(k == 0), stop=(k == DM // 2 - 1),
                                perf_mode=DR)
                        last_gelu = nc.scalar.activation(
                            hT[:, f, :], acc[:], AF.Gelu,
                            bias=b1_sb[:, f:f + 1],
                            scale=1.0 / (WS * WS))
                        if f % 2 == 1:
                            kp = f // 2
                            for ef in range(4):
                                nc.tensor.matmul(
                                    f2acc[ef],
                                    w2ts[ef][:, 2 * kp:2 * kp + 2, :],
                                    hT[:, 2 * kp:2 * kp + 2, :],
                                    start=(kp == 0), stop=(kp == FC // 2 - 1),
                                    perf_mode=DR)
                d3 = nc.scalar.activation(dummy[:], dummy[:], AF.Sqrt)
                add_dep_helper(d3.ins, last_gelu.ins,
                               reason="sqrt preload after last gelu")

                # FFN2 finalize (+ ef 6-7) with LN2 stats interleaved
                z2 = pers.tile([P, DM, NT], BF16, tag="tc")  # xow slot
                s1 = psacc.tile([1, NT], F32, tag="acc")
                s2 = psacc.tile([1, NT], F32, tag="acc")
                for ef in range(DM):
                    if ef < 4:
                        acc_ap = f2acc[ef]
                    else:
                        acc = pspt.tile([P, NT], F32, tag="pt")
                        for k in range(FC // 2):
                            nc.tensor.matmul(
                                acc[:], w2ts[ef][:, 2 * k:2 * k + 2, :],
                                hT[:, 2 * k:2 * k + 2, :],
                                start=(k == 0), stop=(k == FC // 2 - 1),
                                perf_mode=DR)
                        acc_ap = acc[:]
                    nc.vector.scalar_tensor_tensor(
                        z2[:, ef, :], acc_ap, b2_sb[:, ef:ef + 1],
                        xln1[:, ef, :], op0=mybir.AluOpType.add,
                        op1=mybir.AluOpType.add)
                    nc.tensor.matmul(s1[:], ones_col[:], z2[:, ef, :],
                                     start=(ef == 0), stop=(ef == DM - 1))
                    sq = sq_pool.tile([P, NT], BF16, tag="sq")
                    nc.gpsimd.tensor_mul(sq[:], z2[:, ef, :], z2[:, ef, :])
                    nc.tensor.matmul(s2[:], ones_col[:], sq[:],
                                     start=(ef == 0), stop=(ef == DM - 1))
            _w2cm.__exit__(None, None, None)

        # -------- LN2 -> output ------------------------------------------
        with tc.tile_pool(name="outstage", bufs=4) as out_pool:
            yT_r = _rearr(yT_d.ap())

            def write_out(k, cols, t, eng):
                o = out_pool.tile([P, NT // 2], F32)
                eng.tensor_scalar(o[:], t[:],
                                  lnw2_sb[:, k:k + 1],
                                  lnb2_sb[:, k:k + 1],
                                  op0=mybir.AluOpType.mult,
                                  op1=mybir.AluOpType.add)
                q = nc.sync if k % 2 == 0 else nc.scalar
                q.dma_start(yT_r[:, k, cols], o[:])
            ln_apply(z2, write_out, interleave=(s1, s2))  # noqa: F821


def _get_nc():
    global _NC_CACHE
    if _NC_CACHE is None:
        _NC_CACHE = _build_nc()
    return _NC_CACHE


def _tile_w(W, out_cols, scale=WS):
    """[Din, Dout] f32 -> fp8 [Dout//out_cols, 128, Din//128, out_cols]
    so each output-chunk's weights are one contiguous DMA slab.  Weights are
    pre-scaled by `scale` (16): raw magnitudes (~1/sqrt(fan_in), i.e.
    +-0.016..0.031) sit at/below e4m3's min normal 2^-6, where quantization
    is absolute (subnormal quanta) and costs ~6% RMS per element; x16 moves
    them into the normal range (~2.5% RMS).  The kernel folds the
    compensation into free scale slots (exp/gelu scale immediates,
    pre-scaled biases and LN1 affine, and LN scale-invariance)."""
    f8 = mybir.dt.np(FP8)
    Din, Dout = W.shape
    t = (scale * W).astype(f8).reshape(Din // P, P, Dout // out_cols, out_cols)
    return np.ascontiguousarray(t.transpose(2, 1, 0, 3))


def make_in_maps(x, w_qkv, w_out, ln1_w, ln1_b, w1, b1, w2, b2,
                 ln2_w, ln2_b):
    import ml_dtypes
    bf = ml_dtypes.bfloat16
    x = np.ascontiguousarray(np.asarray(x, dtype=np.float32))
    w_qkv = np.asarray(w_qkv, np.float32)
    shared = {
        "w_q": _tile_w(w_qkv[:, 0:D], P),
        "w_k": _tile_w(w_qkv[:, D:2 * D], P),
        "w_v": _tile_w(w_qkv[:, 2 * D:3 * D], 512),
        "w_out": _tile_w(np.asarray(w_out, np.float32), P),
        "w1": _tile_w(np.asarray(w1, np.float32), 512),
        "w2": _tile_w(np.asarray(w2, np.float32), P),
        "b1": np.asarray(b1, np.float32),
        "b2": WS * np.asarray(b2, np.float32),       # spine runs at x16
        "ln1_w": WS * np.asarray(ln1_w, np.float32),  # xln1 carries x16
        "ln1_b": WS * np.asarray(ln1_b, np.float32),
        "ln2_w": np.asarray(ln2_w, np.float32),       # LN2 emits true scale
        "ln2_b": np.asarray(ln2_b, np.float32),
    }
    f8 = mybir.dt.np(FP8)
    in_maps = []
    for c in range(8):
        b, q = divmod(c, 4)
        xT = np.ascontiguousarray(x[b].T)             # [D, NSEQ]
        # rotate so this core's own tokens are always columns [0, NT)
        xTr = np.ascontiguousarray(np.roll(xT, -q * NT, axis=1))
        in_maps.append({
            "xT": np.ascontiguousarray(xTr.astype(f8)),
            "x_own": np.ascontiguousarray(
                (WS * xTr[:, 0:NT]).astype(bf)),      # residual at x16
            **shared,
        })
    return in_maps


def kernel(x, w_qkv, w_out, ln1_w, ln1_b, w1, b1, w2, b2, ln2_w, ln2_b):
    in_maps = make_in_maps(x, w_qkv, w_out, ln1_w, ln1_b, w1, b1, w2, b2,
                           ln2_w, ln2_b)
    nc = _get_nc()
    res = run_bass_kernel_spmd(nc, in_maps, list(range(8)))

    out = np.empty((B, NSEQ, D), np.float32)
    for c in range(8):
        b, q = divmod(c, 4)
        out[b, q * NT:(q + 1) * NT, :] = res.results[c]["yT"].T
    return out



# revision 91
# speedup vs baseline: 1.0071x; 1.0071x over previous
"""Trainium2 Bass kernel for a single-layer transformer encoder.

Model: B=2, N=2048, D=1024, H=16, DFF=4096 (pre-computed QKV attention +
residual/LN + GELU FFN + residual/LN).

Sharding: 2 batches x 4-way sequence split (core c owns the 512 query
tokens q=c%4 of batch b=c//4).  Each core computes the FULL K/V for its
batch (2x redundant vs a pair-split, but it removes every cross-core
collective/exchange and their latency cliffs; the extra PE work rides in
slack).  Rotated token quarters order production q0,q2,q1,q3.

Attention is emitted as eight sequential head-pair sweeps JIT-interleaved
with production: exp(hp, quarter t) needs only K feature chunk kf=hp of t,
Q chunk hp, and V dvc=hp//4 of t, so scores/exp start ~5us in while the
remaining K feature chunks and the V dvc=1 half are produced between
sweeps.  Engine use is balanced so the ~133us of ScalarE exp ([128,1024]
PSUM tiles, denominators from a ones-column in V) and ~140us of PE work
overlap at >80% each.  PSUM is budgeted exactly: pt double-buffer 8KB +
one sweep's oaccs 4KB + production/bc accs 4KB = 16KB.

All dense projections (QKV, attention output, FFN1/FFN2) run as fp8-e4m3
DoubleRow matmuls.  fp8 weights are pre-scaled x16 on the host: raw
magnitudes (~1/sqrt(fan_in)) sit at/below e4m3's min normal where
quantization is absolute, and the rescale roughly halves the end-to-end
error (compensation folds into exp/gelu scale immediates, pre-scaled
biases/LN1 affine, and LN scale-invariance).

Scheduling details the tile framework's greedy ready-list scheduler needs
help with: ACT_TABLE_LOADs are pulled off the critical path by dummy
Sqrt/Gelu activations dep-pinned right after the last exp / LN1-sqrt /
last gelu; w1 prefetch DMAs are emitted between early sweeps (8-slot pool,
no slot-reuse waits) while w2 prefetches ride the idle SP queue gated by
the freed xT/wall pool boundary; FFN2's first 4 output chunks accumulate
k-partials inside the gelu-paced FFN1 window (2 pspt slots as column-pair
accumulators); LN stats are PE ones-column matmuls interleaved into the
producing loops; LN applies split token-halves across DVE and GpSimd.
Output DMA is pipelined across both HWDGE queues.
"""

import os
import sys

for _p in ("/opt/trn_rl_repo", "/root/.axon_site", "/root/.axon_site/_ro/trn_rl_repo"):
    if os.path.isdir(_p) and _p not in sys.path:
        sys.path.append(_p)

import numpy as np

import concourse.bacc as bacc
import concourse.mybir as mybir
import concourse.tile as tile
from concourse.tile_rust import add_dep_helper
from concourse.bass_utils import run_bass_kernel_spmd

P = 128
B, NSEQ, D, H, DFF = 2, 2048, 1024, 16, 4096
DH = D // H                     # 64
NT = 512                        # query tokens per core
DM = D // P                     # 8 feature chunks
JC = NSEQ // P                  # 16 key-token chunks
TC = NSEQ // 512                # 4 512-token chunks
FC = DFF // P                   # 32 FFN feature chunks
HPAIRS = H // 2                 # 8
SCALE = DH ** -0.5
EPS = 1e-5
WS = 16.0                       # fp8 weight pre-scale (see _tile_w)

F32 = mybir.dt.float32
F32R = mybir.dt.float32r
BF16 = mybir.dt.bfloat16
FP8 = mybir.dt.float8e4
I16 = mybir.dt.int16
# Schraudolph exp-approximation constants (DVE path): bf16 bitcast of
# round(x*SEXP_A + SEXP_B) ~= exp(x * SCALE/WS^2); max rel err 3.3%,
# rms 2.1% (c=5.5), only ever applied to a minority of score chunks.
SEXP_A = (SCALE / (WS * WS)) * float(np.log2(np.e)) * 128.0
SEXP_B = 127.0 * 128.0 - 5.5
SEXP_N = int(os.environ.get("SEXP_N", "0"))   # of every 8 chunks, N on DVE
VCP_ACT = os.environ.get("VCP_ACT", "0") == "1"  # V-proj copies on ACT
DR = mybir.MatmulPerfMode.DoubleRow
AF = mybir.ActivationFunctionType

_NC_CACHE = None


def _rearr(ap):
    """DRAM [D_like, T] -> [p, chunk, T] view with chunk-major features."""
    return ap.rearrange("(c p) t -> p c t", p=P)


def _build_nc(reps=1, phases=("qkv", "attn", "proj", "ffn")):
    nc = bacc.Bacc("TRN2", target_bir_lowering=False, debug=False)
    nc.num_devices = 8

    xT = nc.dram_tensor("xT", [D, NSEQ], FP8, kind="ExternalInput")
    x_own = nc.dram_tensor("x_own", [D, NT], BF16, kind="ExternalInput")
    # weights arrive pre-tiled: [out_chunk, partition, in_chunk, out_cols]
    w_q = nc.dram_tensor("w_q", [DM, P, DM, P], FP8, kind="ExternalInput")
    w_k = nc.dram_tensor("w_k", [DM, P, DM, P], FP8, kind="ExternalInput")
    w_v = nc.dram_tensor("w_v", [2, P, DM, 512], FP8, kind="ExternalInput")
    w_out = nc.dram_tensor("w_out", [DM, P, DM, P], FP8, kind="ExternalInput")
    w1 = nc.dram_tensor("w1", [DFF // 512, P, DM, 512], FP8,
                        kind="ExternalInput")
    w2 = nc.dram_tensor("w2", [DM, P, FC, P], FP8, kind="ExternalInput")
    b1 = nc.dram_tensor("b1", [DFF], F32, kind="ExternalInput")
    b2 = nc.dram_tensor("b2", [D], F32, kind="ExternalInput")
    ln1_w = nc.dram_tensor("ln1_w", [D], F32, kind="ExternalInput")
    ln1_b = nc.dram_tensor("ln1_b", [D], F32, kind="ExternalInput")
    ln2_w = nc.dram_tensor("ln2_w", [D], F32, kind="ExternalInput")
    ln2_b = nc.dram_tensor("ln2_b", [D], F32, kind="ExternalInput")
    yT = nc.dram_tensor("yT", [D, NT], F32, kind="ExternalOutput")

    tensors = dict(xT=xT, x_own=x_own, w_q=w_q, w_k=w_k, w_v=w_v, w_out=w_out, w1=w1,
                   w2=w2, b1=b1, b2=b2, ln1_w=ln1_w, ln1_b=ln1_b,
                   ln2_w=ln2_w, ln2_b=ln2_b, yT=yT)

    with tile.TileContext(nc) as tc, \
         nc.allow_low_precision(reason="bf16 matmul operands; fp32 spine"), \
         tc.tile_pool(name="const", bufs=1) as pc:
        C = _emit_consts(nc, pc, tensors)
        for r in range(reps):
            _emit(nc, tc, tensors, C, phases=phases)
    nc.compile()
    return nc


def _emit_consts(nc, pc, T):
    """Constant tiles, emitted ONCE outside the rep loop: re-emitting them
    per rep creates WAR chains from each rep's first memset back to the
    previous rep's very last constant read, serializing reps."""
    ones_f32 = pc.tile([P, 2 * P], F32)
    nc.vector.memset(ones_f32[:], 1.0)
    ones_col = pc.tile([P, 1], BF16)          # lhsT for bf16 stat sums
    nc.vector.tensor_copy(ones_col[:], ones_f32[:, 0:1])
    ones_row = pc.tile([1, P], F32)           # lhsT for exact broadcasts
    nc.vector.tensor_copy(ones_row[:], ones_f32[0:1, 0:P])
    ones_row_b = pc.tile([1, P], BF16)        # bf16 lhsT (full-rate matmul)
    nc.vector.tensor_copy(ones_row_b[:], ones_f32[0:1, 0:P])
    eps_sb = pc.tile([1, 1], F32)
    nc.vector.memset(eps_sb[:], EPS)
    b1_sb = pc.tile([P, FC], F32)
    b2_sb = pc.tile([P, DM], F32)
    lnw1_sb = pc.tile([P, DM], F32)
    lnb1_sb = pc.tile([P, DM], F32)
    lnw2_sb = pc.tile([P, DM], F32)
    lnb2_sb = pc.tile([P, DM], F32)
    dummy = pc.tile([1, 1], F32)
    nc.vector.memset(dummy[:], 1.0)
    for sb, t in ((b1_sb, "b1"), (b2_sb, "b2"),
                  (lnw1_sb, "ln1_w"), (lnb1_sb, "ln1_b"),
                  (lnw2_sb, "ln2_w"), (lnb2_sb, "ln2_b")):
        nc.gpsimd.dma_start(sb[:], T[t].ap().rearrange("(c p) -> p c", p=P))
    return dict(ones_f32=ones_f32, ones_col=ones_col, ones_row=ones_row,
                ones_row_b=ones_row_b, eps_sb=eps_sb, b1_sb=b1_sb,
                b2_sb=b2_sb, lnw1_sb=lnw1_sb, lnb1_sb=lnb1_sb,
                lnw2_sb=lnw2_sb, lnb2_sb=lnb2_sb, dummy=dummy)


def _emit(nc, tc, T, C, phases=("qkv", "attn", "proj", "ffn")):
    xT_d, yT_d = T["xT"], T["yT"]
    ones_f32 = C["ones_f32"]
    ones_col = C["ones_col"]
    ones_row = C["ones_row"]
    ones_row_b = C["ones_row_b"]
    eps_sb = C["eps_sb"]
    b1_sb = C["b1_sb"]
    b2_sb = C["b2_sb"]
    lnw1_sb = C["lnw1_sb"]
    lnb1_sb = C["lnb1_sb"]
    lnw2_sb = C["lnw2_sb"]
    lnb2_sb = C["lnb2_sb"]
    dummy = C["dummy"]

    # ---------------- whole-kernel pools ----------------
    with tc.tile_pool(name="pers", bufs=1) as pers, \
         tc.tile_pool(name="scratch", bufs=4) as sq_pool, \
         tc.tile_pool(name="vecs", bufs=4) as vec_pool, \
         tc.tile_pool(name="psacc", bufs=2, space="PSUM") as psacc, \
         tc.tile_pool(name="pspt", bufs=2, space="PSUM") as pspt, \
         tc.tile_pool(name="psout", bufs=2, space="PSUM") as psout:

        # persistent activations
        QT = pers.tile([P, DM, NT], BF16)
        outT = pers.tile([P, DM, NT], FP8)
        xow = pers.tile([P, DM, NT], BF16, tag="tc")
        xln18 = pers.tile([P, DM, NT], FP8)  # fp8 operand copy of xln1  # own-token x (residual 1)
        xln1 = pers.tile([P, DM, NT], BF16)     # LN1 out (ffn operand+residual)

        def ln_apply(z_tile, writes, interleave=None):
            """LayerNorm over features of z_tile [P, DM, NT] (fp32).
            writes(k, src_f32_ap) stores chunk k.  If interleave is given,
            (s1, s2) stats were already accumulated there by the caller."""
            if interleave is None:
                s1 = psacc.tile([1, NT], F32, tag="acc")
                s2 = psacc.tile([1, NT], F32, tag="acc")
                for k in range(DM):
                    eng = nc.vector if k % 2 == 0 else nc.gpsimd
                    nc.tensor.matmul(s1[:], ones_col[:], z_tile[:, k, :],
                                     start=(k == 0), stop=(k == DM - 1))
                    sq = sq_pool.tile([P, NT], BF16, tag="sq")
                    eng.tensor_mul(sq[:], z_tile[:, k, :], z_tile[:, k, :])
                    nc.tensor.matmul(s2[:], ones_col[:], sq[:],
                                     start=(k == 0), stop=(k == DM - 1))
            else:
                s1, s2 = interleave
            mu = vec_pool.tile([1, NT], F32, tag="v")
            nc.vector.tensor_scalar_mul(mu[:], s1[:], 1.0 / D)
            var = vec_pool.tile([1, NT], F32, tag="v")
            nc.vector.tensor_mul(var[:], mu[:], s1[:])
            nc.vector.tensor_sub(var[:], s2[:], var[:])
            sqrt_ins = nc.scalar.activation(var[:], var[:], AF.Sqrt,
                                            scale=1.0 / D, bias=eps_sb[:])
            ln_apply.last_sqrt = sqrt_ins
            rec = vec_pool.tile([1, NT], F32, tag="v")
            nc.vector.reciprocal(rec[:], var[:])
            murf = vec_pool.tile([1, NT], F32, tag="v")
            nc.vector.tensor_mul(murf[:], mu[:], rec[:])
            R = psacc.tile([P, NT], F32, tag="acc")
            nc.tensor.matmul(R[:], ones_row[:], rec[:],
                             start=True, stop=True)
            MR = psacc.tile([P, NT], F32, tag="acc")
            nc.tensor.matmul(MR[:], ones_row[:], murf[:],
                             start=True, stop=True)
            R_sb = vec_pool.tile([P, NT], F32, tag="v")
            nc.scalar.copy(R_sb[:], R[:])
            MR_sb = vec_pool.tile([P, NT], F32, tag="v")
            nc.scalar.copy(MR_sb[:], MR[:])
            HNT = NT // 2
            for k in range(DM):
                for h in range(2):
                    eng = nc.vector if h == 0 else nc.gpsimd
                    cols = slice(h * HNT, (h + 1) * HNT)
                    t = sq_pool.tile([P, HNT], F32, tag="sq")
                    eng.tensor_mul(t[:], z_tile[:, k, cols], R_sb[:, cols])
                    eng.tensor_sub(t[:], t[:], MR_sb[:, cols])
                    writes(k, cols, t, eng)

        with tc.tile_pool(name="ktp", bufs=1) as kt_pool, \
             tc.tile_pool(name="vpp", bufs=1) as vp_pool, \
             tc.tile_pool(name="pt", bufs=6) as pt_pool, \
             tc.tile_pool(name="wop", bufs=1) as wo_pool, \
             tc.tile_pool(name="w1p", bufs=8) as w1_pool:
            wo_t = wo_pool.tile([P, DM, DM, P], FP8)
            w1ts = []

            def prefetch_w1(lo, hi):
                # Pool-queue DMAs, interleaved between attention sweeps so
                # they run early but never starve a fin's broadcast.
                for fg in range(lo, hi):
                    w1t = w1_pool.tile([P, DM, 512], FP8, tag="w1",
                                       name=f"w1t{fg}")
                    nc.gpsimd.dma_start(w1t[:], T["w1"].ap()[fg])
                    w1ts.append(w1t)

            kt = kt_pool.tile([P, DM, 1024], BF16, tag="kt")
            ktr = kt_pool.tile([P, DM, 1024], BF16, tag="ktr")
            vp = vp_pool.tile([P, 8, H * 65], BF16, tag="vp")
            vpr = vp_pool.tile([P, 8, H * 65], BF16, tag="vpr")
            vp_h = vp.rearrange("p j (h e) -> p j h e", e=65)
            vpr_h = vpr.rearrange("p j (h e) -> p j h e", e=65)
            nc.vector.tensor_copy(
                vp_h[:, :, :, 64:65],
                ones_f32[:, 0:128].rearrange("p (a b c) -> p a b c", b=H, c=1))
            nc.vector.tensor_copy(
                vpr_h[:, :, :, 64:65],
                ones_f32[:, 0:128].rearrange("p (a b c) -> p a b c",
                                             b=H, c=1))

            def kt_at(jc):
                """(tile, column-base) for rotated key chunk jc."""
                if jc < 4: return kt, jc * P
                if jc < 8: return ktr, (jc - 4) * P
                if jc < 12: return kt, (jc - 4) * P
                return ktr, (jc - 8) * P

            def vp_at(jc):
                if jc < 4: return vp, jc
                if jc < 8: return vpr, jc - 4
                if jc < 12: return vp, jc - 4
                return vpr, jc - 8

            def vp_at_h(jc):
                if jc < 4: return vp_h, jc
                if jc < 8: return vpr_h, jc - 4
                if jc < 12: return vp_h, jc - 4
                return vpr_h, jc - 8

            # ---------- attention helpers (emission interleaved below) ----
            oaccs = {}
            last_exp = [None]
            exp_ctr = [0]

            def attn_chunk(hp, jcs, pool, tag, start, stop):
                if start:
                    oaccs[hp] = [pool.tile([65, NT], F32, tag=tag,
                                           name=f"oacc{hp}_{i}")
                                 for i in range(2)]
                oacc = oaccs[hp]
                for n, jc in enumerate(jcs):
                    ksrc, kcb = kt_at(jc)
                    vsrc, vpos = vp_at(jc)
                    pt_ps = pspt.tile([P, 2 * NT], F32, tag="pt")
                    for i in range(2):
                        rows = slice(64 * i, 64 * i + 64)
                        nc.tensor.matmul(
                            pt_ps[:, i * NT:(i + 1) * NT],
                            ksrc[rows, hp, kcb:kcb + P],
                            QT[rows, hp, :],
                            start=True, stop=True)
                    pt_sb = pt_pool.tile([P, 2 * NT], BF16, tag="ptsb")
                    exp_ctr[0] += 1
                    if exp_ctr[0] % 8 < SEXP_N:
                        # DVE Schraudolph exp: offloads the ACT engine (the
                        # attention-phase co-bottleneck) for 3/8 of chunks.
                        nc.vector.tensor_scalar(
                            pt_sb[:].bitcast(I16), pt_ps[:],
                            SEXP_A, SEXP_B,
                            op0=mybir.AluOpType.mult,
                            op1=mybir.AluOpType.add)
                    else:
                        last_exp[0] = nc.scalar.activation(
                            pt_sb[:], pt_ps[:], AF.Exp,
                            scale=SCALE / (WS * WS))
                    for i in range(2):
                        h = 2 * hp + i
                        nc.tensor.matmul(
                            oacc[i][:],
                            vsrc[:, vpos, h * 65:(h + 1) * 65],
                            pt_sb[:, i * NT:(i + 1) * NT],
                            start=(start and n == 0),
                            stop=(stop and n == len(jcs) - 1))

            def attn_fin(hp):
                oacc = oaccs.pop(hp)
                bc2 = psacc.tile([P, NT], F32, tag="acc")
                for i in range(2):
                    # bf16 denominators: 0.4% relative, swamped by the fp8
                    # outT quantization right after; bf16 matmul runs at
                    # full rate and, unlike f32r, may target partition 64.
                    rec = vec_pool.tile([1, NT], BF16, tag="v")
                    nc.vector.reciprocal(rec[:], oacc[i][64:65, :])
                    nc.tensor.matmul(bc2[64 * i:64 * i + 64, :],
                                     ones_row_b[:, 0:64], rec[:],
                                     start=True, stop=True)
                bc_sb = sq_pool.tile([P, NT], F32, tag="sq")
                nc.vector.tensor_copy(bc_sb[:], bc2[:])
                for i in range(2):
                    nc.vector.tensor_mul(
                        outT[64 * i:64 * i + 64, hp, :],
                        oacc[i][0:64, :],
                        bc_sb[64 * i:64 * i + 64, :])

            with tc.tile_pool(name="xpool", bufs=1) as px:
                xT = px.tile([P, DM, NSEQ], FP8)
                xTs = _rearr(xT_d.ap())
                xTq = xTs.rearrange("p c (h q2 t) -> p c h q2 t", h=2, q2=2)
                xTt = xT.rearrange("p c (h q2 t) -> p c h q2 t", h=2, q2=2)
                # quarter order q0, q2, q1, q3 matches K/V production order
                nc.sync.dma_start(xTt[:, :, 0, 0, :], xTq[:, :, 0, 0, :])

                with tc.tile_pool(name="wall", bufs=1) as wall_pool:
                    # wk on SP right after xT-q0; wq alone on ACT so the
                    # QT copies (ACT) can start ~4us in, pulling the first
                    # exp forward.
                    wk = wall_pool.tile([P, DM, DM, P], FP8, tag="wk")
                    wks = T["w_k"].ap().rearrange("f p k t -> p f k t")
                    nc.sync.dma_start(wk[:, 0:4], wks[:, 0:4])
                    nc.sync.dma_start(wk[:, 4:8], wks[:, 4:8])
                    wq = wall_pool.tile([P, DM, DM, P], FP8, tag="wq")
                    wqs = T["w_q"].ap().rearrange("f p k t -> p f k t")
                    nc.scalar.dma_start(wq[:, 0:4], wqs[:, 0:4])
                    nc.scalar.dma_start(wq[:, 4:8], wqs[:, 4:8])
                    nc.sync.dma_start(xTt[:, :, 1, 0, :], xTq[:, :, 1, 0, :])
                    nc.sync.dma_start(xTt[:, :, 0, 1, :], xTq[:, :, 0, 1, :])
                    nc.sync.dma_start(xTt[:, :, 1, 1, :], xTq[:, :, 1, 1, :])
                    wos = T["w_out"].ap().rearrange("f p k t -> p f k t")
                    nc.sync.dma_start(wo_t[:, 0:4], wos[:, 0:4])
                    nc.sync.dma_start(wo_t[:, 4:8], wos[:, 4:8])
                    wv = wall_pool.tile([P, 2, DM, 512], FP8, tag="wv")
                    wvs = T["w_v"].ap().rearrange("f p k t -> p f k t")
                    nc.gpsimd.dma_start(wv[:, 0:1], wvs[:, 0:1])
                    nc.gpsimd.dma_start(wv[:, 1:2], wvs[:, 1:2])
                    last_xow_cp = nc.gpsimd.dma_start(
                        xow[:], _rearr(T["x_own"].ap()))

                    def k_quarter(t, kfs):
                        """K^T chunks kfs for rotated token quarter t."""
                        last = None
                        dst, cb = kt_at(t * 4)
                        for kf in kfs:
                            acc = psacc.tile([P, 512], F32, tag="acc")
                            for k in range(DM // 2):
                                last = nc.tensor.matmul(
                                    acc[:], wk[:, kf, 2 * k:2 * k + 2, :],
                                    xT[:, 2 * k:2 * k + 2,
                                       t * 512:(t + 1) * 512],
                                    start=(k == 0), stop=(k == DM // 2 - 1),
                                    perf_mode=DR)
                            nc.vector.tensor_copy(
                                dst[:, kf, cb:cb + 512], acc[:])
                        return last

                    def k_feat(kf):
                        """K^T feature chunk kf (head pair kf), all quarters."""
                        for t in (0, 2, 1, 3):
                            last = k_quarter(t, [kf])
                        return last

                    def v_quarter(t, dvcs):
                        """V for rotated token quarter t, head halves dvcs."""
                        last = None
                        for dvc in dvcs:
                            for jc4 in range(4):
                                jc = t * 4 + jc4
                                acc = psacc.tile([P, 512], F32, tag="acc")
                                for k in range(DM // 2):
                                    last = nc.tensor.matmul(
                                        acc[:],
                                        xT[:, 2 * k:2 * k + 2,
                                           jc * P:(jc + 1) * P],
                                        wv[:, dvc, 2 * k:2 * k + 2, :],
                                        start=(k == 0),
                                        stop=(k == DM // 2 - 1),
                                        perf_mode=DR)
                                dvh, jpos = vp_at_h(jc)
                                dst = dvh[:, jpos, dvc * 8:(dvc + 1) * 8, 0:64]
                                src = acc[:].rearrange("p (h e) -> p h e",
                                                       e=64)
                                if VCP_ACT:
                                    nc.scalar.copy(dst, src)
                                else:
                                    nc.vector.tensor_copy(dst, src)
                        return last

                    # Phase 1 JIT-interleaved with the attention sweeps.
                    # exp(hp, quarter t) needs only K chunk kf=hp of t, Q
                    # chunk hp, and AV needs V dvc=hp//4 of t — so head-pair
                    # sweeps run sequentially while the remaining K feature
                    # chunks / V head-halves are produced between sweeps.
                    JC_QS = ([0, 1, 2, 3], [8, 9, 10, 11],
                             [4, 5, 6, 7], [12, 13, 14, 15])
                    atn = "attn" in phases

                    def sweep(hp, part):  # part 0: quarters q0+q2; 1: q1+q3
                        if not atn:
                            return
                        attn_chunk(hp, JC_QS[2 * part] + JC_QS[2 * part + 1],
                                   psout, "o", start=(part == 0),
                                   stop=(part == 1))
                        if part == 1:
                            attn_fin(hp)

                    def q_proj(qfs):
                        for qf in qfs:
                            acc = psacc.tile([P, NT], F32, tag="acc")
                            for k in range(DM // 2):
                                nc.tensor.matmul(
                                    acc[:], wq[:, qf, 2 * k:2 * k + 2, :],
                                    xT[:, 2 * k:2 * k + 2, 0:NT],
                                    start=(k == 0), stop=(k == DM // 2 - 1),
                                    perf_mode=DR)
                            nc.scalar.copy(QT[:, qf, :], acc[:])

                    k_quarter(0, [0])
                    q_proj(range(DM))
                    v_quarter(0, (0,))
                    if atn:
                        attn_chunk(0, JC_QS[0], psout, "o", True, False)
                    k_quarter(2, [0])
                    v_quarter(2, (0,))
                    if atn:
                        attn_chunk(0, JC_QS[1], psout, "o", False, False)
                    k_quarter(1, [0])
                    v_quarter(1, (0,))
                    if atn:
                        attn_chunk(0, JC_QS[2], psout, "o", False, False)
                    k_quarter(3, [0])
                    v_quarter(3, (0,))
                    if atn:
                        attn_chunk(0, JC_QS[3], psout, "o", False, True)
                    k_feat(1)   # before fin0: its psacc accs beat fin0's bc2
                    if atn:
                        attn_fin(0)
                    if "ffn" in phases:
                        prefetch_w1(0, 4)
                    sweep(1, 0)
                    if "ffn" in phases:
                        prefetch_w1(4, 8)
                    k_feat(2)
                    sweep(1, 1)
                    k_feat(3)
                    sweep(2, 0)
                    v_quarter(0, (1,))
                    v_quarter(2, (1,))
                    sweep(2, 1)
                    v_quarter(1, (1,))
                    v_quarter(3, (1,))
                    sweep(3, 0)
                    k_feat(4)
                    sweep(3, 1)
                    k_feat(5)
                    sweep(4, 0)
                    k_feat(6)
                    sweep(4, 1)
                    k_feat(7)
                    sweep(5, 0)
                    sweep(5, 1)
                    sweep(6, 0)
                    sweep(6, 1)
                    sweep(7, 0)
                    sweep(7, 1)

            # -------- prefetch FFN2 weights (SP queue; reuses the freed
            # xT/wall SBUF region, so the pool boundary gates these DMAs
            # behind the last phase-1 reads) ------------------------------
            _w2cm = tc.tile_pool(name="w2p", bufs=8)
            w2_pool = _w2cm.__enter__()
            w2ts = []
            if "ffn" in phases:
                for ef in range(DM):
                    w2t = w2_pool.tile([P, FC, P], FP8, tag="w2",
                                       name=f"w2t{ef}")
                    nc.sync.dma_start(w2t[:], T["w2"].ap()[ef])
                    w2ts.append(w2t)

            # -------- attention emitted inside phase 1 above -------------
            if "attn" not in phases:      # timing-bisect stub
                for k in range(DM):
                    nc.vector.tensor_copy(outT[:, k, :], QT[:, k, :])

            # -------- output projection + residual 1 + LN1 stats ---------
            # Pull the Sqrt table in right after the last exp, off the
            # critical path (LN1's Sqrt would otherwise eat the 1.28us
            # ACT_TABLE_LOAD inline).  The dep pins it there — the tile
            # scheduler is a greedy ready-list and would otherwise hoist it
            # into an early idle slot where the exps evict it again.
            d1 = nc.scalar.activation(dummy[:], dummy[:], AF.Sqrt)
            if "attn" in phases:
                add_dep_helper(d1.ins, last_exp[0].ins,
                               reason="sqrt preload after last exp")
            z1 = kt_pool.tile([P, DM, NT], BF16, tag="kt")  # reuses kt slot
            s1 = psacc.tile([1, NT], F32, tag="acc")
            s2 = psacc.tile([1, NT], F32, tag="acc")
            for ef in range(DM):
                acc = pspt.tile([P, NT], F32, tag="pt")
                for k in range(DM // 2):
                    nc.tensor.matmul(acc[:], wo_t[:, ef, 2 * k:2 * k + 2, :],
                                     outT[:, 2 * k:2 * k + 2, :],
                                     start=(k == 0),
                                     stop=(k == DM // 2 - 1),
                                     perf_mode=DR)
                nc.vector.scalar_tensor_tensor(
                    z1[:, ef, :], acc[:], 1.0 / WS, xow[:, ef, :],
                    op0=mybir.AluOpType.mult, op1=mybir.AluOpType.add)
                nc.tensor.matmul(s1[:], ones_col[:], z1[:, ef, :],
                                 start=(ef == 0), stop=(ef == DM - 1))
                sq = sq_pool.tile([P, NT], BF16, tag="sq")
                nc.gpsimd.tensor_mul(sq[:], z1[:, ef, :], z1[:, ef, :])
                nc.tensor.matmul(s2[:], ones_col[:], sq[:],
                                 start=(ef == 0), stop=(ef == DM - 1))

            # -------- LN1 ------------------------------------------------
            def write_xln1(k, cols, t, eng):
                eng.tensor_scalar(xln1[:, k, cols], t[:],
                                  lnw1_sb[:, k:k + 1],
                                  lnb1_sb[:, k:k + 1],
                                  op0=mybir.AluOpType.mult,
                                  op1=mybir.AluOpType.add)
                if cols.start != 0:     # second half done: copy whole chunk
                    if k % 2 == 0:
                        nc.scalar.copy(xln18[:, k, :], xln1[:, k, :])
                    else:
                        nc.vector.tensor_copy(xln18[:, k, :], xln1[:, k, :])
            ln_apply(z1, write_xln1, interleave=(s1, s2))
            if "ffn" in phases:   # preload Gelu table behind LN1's Sqrt
                d2 = nc.scalar.activation(dummy[:], dummy[:], AF.Gelu)
                add_dep_helper(d2.ins, ln_apply.last_sqrt.ins,
                               reason="gelu preload after LN1 sqrt")

            if "ffn" not in phases:   # timing-bisect stub: LN2 input
                z2 = pers.tile([P, DM, NT], BF16, tag="tc")  # xow slot
                s1 = psacc.tile([1, NT], F32, tag="acc")
                s2 = psacc.tile([1, NT], F32, tag="acc")
                for k in range(DM):
                    eng = nc.vector if k % 2 == 0 else nc.gpsimd
                    nc.vector.tensor_copy(z2[:, k, :], z1[:, k, :])
                    nc.tensor.matmul(s1[:], ones_col[:], z2[:, k, :],
                                     start=(k == 0), stop=(k == DM - 1))
                    sq = sq_pool.tile([P, NT], BF16, tag="sq")
                    eng.tensor_mul(sq[:], z2[:, k, :], z2[:, k, :])
                    nc.tensor.matmul(s2[:], ones_col[:], sq[:],
                                     start=(k == 0), stop=(k == DM - 1))

            # -------- FFN ------------------------------------------------
            if "ffn" in phases:
                hT = kt_pool.tile([P, FC, NT], FP8, tag="kt")  # kt/z1 slot
                # FFN2 accumulators for ef 0-5 live across FFN1 (2 pspt
                # slots hold 2 accs each in column halves + 2 psout slots):
                # their k-partials are emitted as hT chunk pairs land, so
                # most of FFN2's PE work hides under the gelu-paced FFN1.
                a01 = pspt.tile([P, 2 * NT], F32, tag="pt", name="f2a01")
                a23 = pspt.tile([P, 2 * NT], F32, tag="pt", name="f2a23")
                f2acc = [a01[:, 0:NT], a01[:, NT:2 * NT],
                         a23[:, 0:NT], a23[:, NT:2 * NT]]
                for fg in range(DFF // 512):
                    w1t = w1ts[fg]
                    for f4 in range(4):
                        f = fg * 4 + f4
                        fpool = psacc if f % 2 == 0 else psout
                        ftag = "acc" if f % 2 == 0 else "o"
                        acc = fpool.tile([P, NT], F32, tag=ftag)
                        for k in range(DM // 2):
                            nc.tensor.matmul(
                                acc[:],
                                w1t[:, 2 * k:2 * k + 2, f4 * P:(f4 + 1) * P],
                                xln18[:, 2 * k:2 * k + 2, :],
                                start=(k == 0), stop=(k == DM // 2 - 1),
                                perf_mode=DR)
                        last_gelu = nc.scalar.activation(
                            hT[:, f, :], acc[:], AF.Gelu,
                            bias=b1_sb[:, f:f + 1],
                            scale=1.0 / (WS * WS))
                        if f % 2 == 1:
                            kp = f // 2
                            for ef in range(4):
                                nc.tensor.matmul(
                                    f2acc[ef],
                                    w2ts[ef][:, 2 * kp:2 * kp + 2, :],
                                    hT[:, 2 * kp:2 * kp + 2, :],
                                    start=(kp == 0), stop=(kp == FC // 2 - 1),
                                    perf_mode=DR)
                d3 = nc.scalar.activation(dummy[:], dummy[:], AF.Sqrt)
                add_dep_helper(d3.ins, last_gelu.ins,
                               reason="sqrt preload after last gelu")

                # FFN2 finalize (+ ef 6-7) with LN2 stats interleaved
                z2 = pers.tile([P, DM, NT], BF16, tag="tc")  # xow slot
                s1 = psacc.tile([1, NT], F32, tag="acc")
                s2 = psacc.tile([1, NT], F32, tag="acc")
                for ef in range(DM):
                    if ef < 4:
                        acc_ap = f2acc[ef]
                    else:
                        acc = pspt.tile([P, NT], F32, tag="pt")
                        for k in range(FC // 2):
                            nc.tensor.matmul(
                                acc[:], w2ts[ef][:, 2 * k:2 * k + 2, :],
                                hT[:, 2 * k:2 * k + 2, :],
                                start=(k == 0), stop=(k == FC // 2 - 1),
                                perf_mode=DR)
                        acc_ap = acc[:]
                    nc.vector.scalar_tensor_tensor(
                        z2[:, ef, :], acc_ap, b2_sb[:, ef:ef + 1],
                        xln1[:, ef, :], op0=mybir.AluOpType.add,
                        op1=mybir.AluOpType.add)
                    nc.tensor.matmul(s1[:], ones_col[:], z2[:, ef, :],
                                     start=(ef == 0), stop=(ef == DM - 1))
                    sq = sq_pool.tile([P, NT], BF16, tag="sq")
                    nc.gpsimd.tensor_mul(sq[:], z2[:, ef, :], z2[:, ef, :])
                    nc.tensor.matmul(s2[:], ones_col[:], sq[:],
                                     start=(ef == 0), stop=(ef == DM - 1))
            _w2cm.__exit__(None, None, None)

        # -------- LN2 -> output ------------------------------------------
        with tc.tile_pool(name="outstage", bufs=4) as out_pool:
            yT_r = _rearr(yT_d.ap())

            def write_out(k, cols, t, eng):
                o = out_pool.tile([P, NT // 2], F32)
                eng.tensor_scalar(o[:], t[:],
                                  lnw2_sb[:, k:k + 1],
                                  lnb2_sb[:, k:k + 1],
                                  op0=mybir.AluOpType.mult,
                                  op1=mybir.AluOpType.add)
                q = nc.sync if k % 2 == 0 else nc.scalar
                q.dma_start(yT_r[:, k, cols], o[:])
            ln_apply(z2, write_out, interleave=(s1, s2))  # noqa: F821


def _get_nc():
    global _NC_CACHE
    if _NC_CACHE is None:
        _NC_CACHE = _build_nc()
    return _NC_CACHE


def _tile_w(W, out_cols, scale=WS):
    """[Din, Dout] f32 -> fp8 [Dout//out_cols, 128, Din//128, out_cols]
    so each output-chunk's weights are one contiguous DMA slab.  Weights are
    pre-scaled by `scale` (16): raw magnitudes (~1/sqrt(fan_in), i.e.
    +-0.016..0.031) sit at/below e4m3's min normal 2^-6, where quantization
    is absolute (subnormal quanta) and costs ~6% RMS per element; x16 moves
    them into the normal range (~2.5% RMS).  The kernel folds the
    compensation into free scale slots (exp/gelu scale immediates,
    pre-scaled biases and LN1 affine, and LN scale-invariance)."""
    f8 = mybir.dt.np(FP8)
    Din, Dout = W.shape
    t = (scale * W).astype(f8).reshape(Din // P, P, Dout // out_cols, out_cols)
    return np.ascontiguousarray(t.transpose(2, 1, 0, 3))


def make_in_maps(x, w_qkv, w_out, ln1_w, ln1_b, w1, b1, w2, b2,
                 ln2_w, ln2_b):
    import ml_dtypes
    bf = ml_dtypes.bfloat16
    x = np.ascontiguousarray(np.asarray(x, dtype=np.float32))
    w_qkv = np.asarray(w_qkv, np.float32)
    shared = {
        "w_q": _tile_w(w_qkv[:, 0:D], P),
        "w_k": _tile_w(w_qkv[:, D:2 * D], P),
        "w_v": _tile_w(w_qkv[:, 2 * D:3 * D], 512),
        "w_out": _tile_w(np.asarray(w_out, np.float32), P),
        "w1": _tile_w(np.asarray(w1, np.float32), 512),
        "w2": _tile_w(np.asarray(w2, np.float32), P),
        "b1": np.asarray(b1, np.float32),
        "b2": WS * np.asarray(b2, np.float32),       # spine runs at x16
        "ln1_w": WS * np.asarray(ln1_w, np.float32),  # xln1 carries x16
        "ln1_b": WS * np.asarray(ln1_b, np.float32),
        "ln2_w": np.asarray(ln2_w, np.float32),       # LN2 emits true scale
        "ln2_b": np.asarray(ln2_b, np.float32),
    }
    f8 = mybir.dt.np(FP8)
    in_maps = []
    for c in range(8):
        b, q = divmod(c, 4)
        xT = np.ascontiguousarray(x[b].T)             # [D, NSEQ]
        # rotate so this core's own tokens are always columns [0, NT)
        xTr = np.ascontiguousarray(np.roll(xT, -q * NT, axis=1))
        in_maps.append({
            "xT": np.ascontiguousarray(xTr.astype(f8)),
            "x_own": np.ascontiguousarray(
                (WS * xTr[:, 0:NT]).astype(bf)),      # residual at x16
            **shared,
        })
    return in_maps


def kernel(x, w_qkv, w_out, ln1_w, ln1_b, w1, b1, w2, b2, ln2_w, ln2_b):
    in_maps = make_in_maps(x, w_qkv, w_out, ln1_w, ln1_b, w1, b1, w2, b2,
                           ln2_w, ln2_b)
    nc = _get_nc()
    res = run_bass_kernel_spmd(nc, in_maps, list(range(8)))

    out = np.empty((B, NSEQ, D), np.float32)
    for c in range(8):
        b, q = divmod(c, 4)
        out[b, q * NT:(q + 1) * NT, :] = res.results[c]["yT"].T
    return out

